# revision 8
# baseline (speedup 1.0000x reference)
"""AmplitudeWeightedPhaseAttention Trainium2 kernel (8 NeuronCores, SPMD).

Math: the reference's [B,Sq,Sk,F] tensor collapses algebraically.
With rfft bin features re/im and amp2 = re^2 + im^2:
    t  = amp2^(-1/4)       u = re*t   v = im*t   w = amp2^(+1/4)
    num[i,j] = sum_f u_q u_k + v_q v_k        (v==0 at f=0 and f=64)
    den[i,j] = sum_f w_q w_k                  (rank-65 matmul)
    weights  = softmax_j(num/den + 1)         out = weights @ V
Sharding: core c owns batch c//4, query rows (c%4)*256..+256.  Scores are
computed in natural [i_p, j] layout (softmax normalization is a per-
partition scalar); the normalized weights are PE-transposed to feed P@V.
"""

import numpy as np
from contextlib import ExitStack

B, S, H = 2, 1024, 128
F = H // 2 + 1  # 65
NCORES = 8
QBLK = S // 4   # 256 query rows per core
NJT = S // 128  # 8 key tiles
NIT = QBLK // 128  # 2 query sub-tiles
NF = S + QBLK  # feature columns: K cols then Q cols

WEIGHTS_BF16 = True  # store weights output as bf16, upcast on host

_CACHE = {}


def _dft_consts():
    h = np.arange(H, dtype=np.float64)[:, None]
    f = np.arange(F, dtype=np.float64)[None, :]
    C = np.cos(2 * np.pi * h * f / H)
    Sn = -np.sin(2 * np.pi * h * f / H)
    Sn[:, 0] = 0.0
    Sn[:, F - 1] = 0.0  # exactly zero at DC and Nyquist
    return np.concatenate([C, Sn], axis=1).astype(np.float32)  # [128, 130]


def _patch_act_tables():
    """Make Ln and Exp both first-match to natural_log_exp_and_others so the
    whole kernel needs a single ACT table load. Set IDs stay canonical (we
    only edit membership of the shadowing sets, keyed by name)."""
    import concourse.bacc as bacc_mod
    from concourse import mybir
    if getattr(bacc_mod, "_awpa_tables_patched", False):
        return
    orig = bacc_mod.get_activation_tables
    AF = mybir.ActivationFunctionType

    def patched(arch):
        tables = dict(orig(arch))
        if "natural_log_exp_and_others" in tables:
            keep = tables["natural_log_exp_and_others"]
            for name, fns in tables.items():
                if name == "natural_log_exp_and_others":
                    continue
                if AF.Ln in fns or AF.Exp in fns:
                    tables[name] = fns - {AF.Ln, AF.Exp}
        return tables

    bacc_mod.get_activation_tables = patched
    bacc_mod._awpa_tables_patched = True


def _build():
    import concourse.bass as bass
    import concourse.tile as tile
    from concourse import bacc, mybir

    _patch_act_tables()
    f32 = mybir.dt.float32
    bf16 = mybir.dt.bfloat16
    AF = mybir.ActivationFunctionType

    nc = bacc.Bacc("TRN2", target_bir_lowering=False, debug=False,
                   num_devices=NCORES)
    Qs = nc.dram_tensor("Qs", [QBLK, H], f32, kind="ExternalInput").ap()
    K = nc.dram_tensor("K", [S, H], f32, kind="ExternalInput").ap()
    V = nc.dram_tensor("V", [S, H], f32, kind="ExternalInput").ap()
    CD = nc.inline_tensor(_dft_consts(), "CDconst").ap()  # [128, 130]
    IDT = nc.inline_tensor(np.eye(H, dtype=np.float32), "IDTconst").ap()
    ow_dt = bf16 if WEIGHTS_BF16 else f32
    OW = nc.dram_tensor("OW", [QBLK, S], ow_dt, kind="ExternalOutput").ap()
    OO = nc.dram_tensor("OO", [H, QBLK], f32, kind="ExternalOutput").ap()

    with ExitStack() as ctx:
        tc = ctx.enter_context(tile.TileContext(nc))
        consts = ctx.enter_context(tc.tile_pool(name="consts", bufs=1))
        big = ctx.enter_context(tc.tile_pool(name="big", bufs=1))
        ftmp = ctx.enter_context(tc.tile_pool(name="ftmp", bufs=1))

        # ACT table preload: first ACT op is Ln -> loads the (patched) set
        # covering Ln/Exp/Square/Copy once, overlapping the input DMAs.
        junk = consts.tile([128, 1], f32)
        nc.vector.memset(junk[:], 1.0)
        junk2 = consts.tile([128, 1], f32)
        nc.scalar.activation(junk2[:], junk[:], AF.Ln)

        # --- DMA inputs (criticality order) ---
        kn = big.tile([128, NJT, H], f32)
        nc.sync.dma_start(out=kn[:], in_=K.rearrange("(t p) h -> p t h", p=128))
        idt = consts.tile([H, H], f32)
        nc.sync.dma_start(out=idt[:], in_=IDT[:])
        qn = big.tile([128, NIT, H], f32)
        nc.sync.dma_start(out=qn[:], in_=Qs.rearrange("(t p) h -> p t h", p=128))
        cdf = consts.tile([H, 2 * F], f32)
        nc.sync.dma_start(out=cdf[:], in_=CD[:])
        vn = big.tile([128, NJT, H], f32)
        nc.sync.dma_start(out=vn[:], in_=V.rearrange("(t p) h -> p t h", p=128))
        cdb = consts.tile([H, 2 * F], bf16)
        nc.vector.tensor_copy(cdb[:], cdf[:])
        vb = big.tile([128, NJT, H], bf16)
        nc.vector.tensor_copy(vb[:], vn[:])

        # --- input transposes (h on partitions) via PE ---
        with tc.tile_pool(name="psA", bufs=1, space="PSUM") as psA:
            kt_ps = psA.tile([128, S], f32, tag="kt")
            for t in range(NJT):
                nc.tensor.transpose(kt_ps[:, t * 128:(t + 1) * 128],
                                    kn[:, t, :], idt[:])
            ktb = big.tile([128, S], bf16)
            nc.vector.tensor_copy(ktb[:], kt_ps[:])
            qt_ps = psA.tile([128, QBLK], f32, tag="qt")
            for t in range(NIT):
                nc.tensor.transpose(qt_ps[:, t * 128:(t + 1) * 128],
                                    qn[:, t, :], idt[:])
            qtb = big.tile([128, QBLK], bf16)
            nc.vector.tensor_copy(qtb[:], qt_ps[:])

        # --- DFT + features, K and Q batched along free dim [65, 1280] ---
        uF = big.tile([F, NF], bf16)       # u features (cols: K | Q)
        vF = big.tile([F - 1, NF], bf16)   # v features f=0..63 (v(0)=0)
        wF = big.tile([F, NF], bf16)       # w features
        with tc.tile_pool(name="psB", bufs=1, space="PSUM") as psB:
            re_ps = psB.tile([F, NF], f32, tag="re")
            im_ps = psB.tile([F, NF], f32, tag="im")
            for c0, c1, src, s0 in ((0, 512, ktb, 0), (512, 1024, ktb, 512),
                                    (1024, NF, qtb, 0)):
                nc.tensor.matmul(re_ps[:, c0:c1], cdb[:, 0:F],
                                 src[:, s0:s0 + (c1 - c0)],
                                 start=True, stop=True)
                nc.tensor.matmul(im_ps[:, c0:c1], cdb[:, F:2 * F],
                                 src[:, s0:s0 + (c1 - c0)],
                                 start=True, stop=True)
            sqre = ftmp.tile([F, NF], bf16)
            nc.scalar.activation(sqre[:], re_ps[:], AF.Square)
            sqim = ftmp.tile([F, NF], bf16)
            nc.scalar.activation(sqim[:], im_ps[:], AF.Square)
            amp2 = ftmp.tile([F, NF], bf16)
            nc.vector.tensor_add(amp2[:], sqre[:], sqim[:])
            lg = ftmp.tile([F, NF], f32)
            nc.scalar.activation(lg[:], amp2[:], AF.Ln)
            tq = ftmp.tile([F, NF], f32)
            nc.scalar.activation(tq[:], lg[:], AF.Exp, scale=-0.25)
            nc.scalar.activation(wF[:], lg[:], AF.Exp, scale=0.25)
            nc.vector.tensor_mul(uF[:], re_ps[:], tq[:])
            nc.vector.tensor_mul(vF[:], im_ps[0:F - 1, :], tq[0:F - 1, :])

        # --- scores + softmax per query sub-tile (natural [i_p, j]) ---
        KO = S  # Q-feature column offset in uF/vF/wF
        wb_tiles = []
        with tc.tile_pool(name="psC", bufs=2, space="PSUM") as psC, \
             tc.tile_pool(name="psD", bufs=2, space="PSUM") as psD:
            for it in range(NIT):  # noqa: E301
                i0, i1 = KO + it * 128, KO + (it + 1) * 128
                num_ps = psC.tile([128, S], f32, tag="num")
                den_ps = psD.tile([128, S], f32, tag="den")
                for c0 in range(0, S, 512):
                    c1 = c0 + 512
                    nc.tensor.matmul(num_ps[:, c0:c1], uF[:, i0:i1],
                                     uF[:, c0:c1], start=True, stop=False)
                    nc.tensor.matmul(num_ps[:, c0:c1], vF[:, i0:i1],
                                     vF[:, c0:c1], start=False, stop=True)
                    nc.tensor.matmul(den_ps[:, c0:c1], wF[:, i0:i1],
                                     wF[:, c0:c1], start=True, stop=True)
                inv = ftmp.tile([128, S], f32, tag="inv")
                nc.vector.reciprocal_approx_fast(out=inv[:], in_=den_ps[:])
                pa = ftmp.tile([128, S], f32, tag="pa")
                nc.vector.tensor_mul(pa[:], num_ps[:], inv[:])
                e = ftmp.tile([128, S], bf16, tag="e")
                sumexp = ftmp.tile([128, 1], f32, tag="sumexp")
                nc.scalar.activation(e[:], pa[:], AF.Exp, bias=1.0,
                                     accum_out=sumexp[:])
                r = ftmp.tile([128, 1], f32, tag="r")
                nc.vector.reciprocal(r[:], sumexp[:])
                wb = big.tile([128, S], bf16, tag=f"wb{it}")
                nc.vector.tensor_scalar_mul(wb[:], e[:], r[:])
                wb_tiles.append(wb)
                if WEIGHTS_BF16:
                    nc.sync.dma_start(out=OW[it * 128:(it + 1) * 128, :],
                                      in_=wb[:])
                else:
                    oww = ftmp.tile([128, S], f32, tag="oww")
                    nc.vector.tensor_scalar_mul(oww[:], e[:], r[:])
                    nc.sync.dma_start(out=OW[it * 128:(it + 1) * 128, :],
                                      in_=oww[:])

        # --- transpose normalized weights to [j_p, i] for P@V ---
        idtb = consts.tile([H, H], bf16)
        nc.vector.tensor_copy(idtb[:], idt[:])
        with tc.tile_pool(name="psE", bufs=2, space="PSUM") as psE:
            et_g = []
            for g in range(2):  # 4 j-tiles per psum group, one batched copy
                wt_ps = psE.tile([128, 4 * QBLK], bf16, tag="wt")
                for lj in range(4):
                    jt = g * 4 + lj
                    for it in range(NIT):
                        nc.tensor.transpose(
                            wt_ps[:, lj * QBLK + it * 128:
                                  lj * QBLK + (it + 1) * 128],
                            wb_tiles[it][:, jt * 128:(jt + 1) * 128], idtb[:])
                et = big.tile([128, 4, QBLK], bf16, tag=f"et{g}")
                nc.scalar.copy(et[:], wt_ps[:])
                et_g.append(et)

            # --- P@V: out^T[h, i] accumulated over j tiles ---
            av_ps = psE.tile([128, QBLK], f32, tag="av")
            for jt in range(NJT):
                nc.tensor.matmul(av_ps[:], vb[:, jt, :],
                                 et_g[jt // 4][:, jt % 4, :],
                                 start=(jt == 0), stop=(jt == NJT - 1))
            oo = big.tile([H, QBLK], f32)
            nc.vector.tensor_copy(oo[:], av_ps[:])
            nc.sync.dma_start(out=OO[:], in_=oo[:])

    nc.compile()
    return nc


def _get_nc():
    if "nc" not in _CACHE:
        _CACHE["nc"] = _build()
    return _CACHE["nc"]


def kernel(Q, K, V):
    from concourse.bass_utils import run_bass_kernel_spmd

    Q = np.ascontiguousarray(np.asarray(Q, dtype=np.float32))
    K = np.ascontiguousarray(np.asarray(K, dtype=np.float32))
    V = np.ascontiguousarray(np.asarray(V, dtype=np.float32))
    nc = _get_nc()
    in_maps = []
    for c in range(NCORES):
        b, qb = c // 4, c % 4
        in_maps.append({
            "Qs": np.ascontiguousarray(Q[b, qb * QBLK:(qb + 1) * QBLK]),
            "K": K[b],
            "V": V[b],
        })
    res = run_bass_kernel_spmd(nc, in_maps, core_ids=list(range(NCORES)))
    output = np.empty((B, S, H), np.float32)
    weights = np.empty((B, S, S), np.float32)
    for c in range(NCORES):
        b, qb = c // 4, c % 4
        rr = res.results[c]
        weights[b, qb * QBLK:(qb + 1) * QBLK, :] = rr["OW"].astype(np.float32)
        output[b, qb * QBLK:(qb + 1) * QBLK, :] = rr["OO"].T
    return output, weights


if __name__ == "__main__":
    rng = np.random.default_rng(0)
    Q = rng.standard_normal((B, S, H)).astype(np.float32)
    K = rng.standard_normal((B, S, H)).astype(np.float32)
    V = rng.standard_normal((B, S, H)).astype(np.float32)
    out, w = kernel(Q, K, V)
    print("kernel ran:", out.shape, w.shape)


# revision 11
# speedup vs baseline: 1.0108x; 1.0108x over previous
"""AmplitudeWeightedPhaseAttention Trainium2 kernel (8 NeuronCores, SPMD).

Math: the reference's [B,Sq,Sk,F] tensor collapses algebraically.
With rfft bin features re/im and amp2 = re^2 + im^2:
    t  = amp2^(-1/4)       u = re*t   v = im*t   w = amp2^(+1/4)
    num[i,j] = sum_f u_q u_k + v_q v_k        (v==0 at f=0 and f=64)
    den[i,j] = sum_f w_q w_k                  (rank-65 matmul)
    weights  = softmax_j(num/den + 1)         out = weights @ V
Sharding: core c owns batch c//4, query rows (c%4)*256..+256.  Scores are
computed in natural [i_p, j] layout (softmax normalization is a per-
partition scalar); normalized weights are PE-transposed to feed P@V.
Feature columns are laid out [Q | K] and processed in two chunks so the
ACT/DVE/PE stages pipeline chunk-against-chunk.
"""

import numpy as np
from contextlib import ExitStack

B, S, H = 2, 1024, 128
F = H // 2 + 1  # 65
NCORES = 8
QBLK = S // 4   # 256 query rows per core
NJT = S // 128  # 8 key tiles
NIT = QBLK // 128  # 2 query sub-tiles
NF = QBLK + S   # feature columns: Q cols [0:256], K cols [256:1280]

WEIGHTS_BF16 = True  # store weights output as bf16, upcast on host

_CACHE = {}


def _dft_consts():
    h = np.arange(H, dtype=np.float64)[:, None]
    f = np.arange(F, dtype=np.float64)[None, :]
    C = np.cos(2 * np.pi * h * f / H)
    Sn = -np.sin(2 * np.pi * h * f / H)
    Sn[:, 0] = 0.0
    Sn[:, F - 1] = 0.0  # exactly zero at DC and Nyquist
    return np.concatenate([C, Sn], axis=1).astype(np.float32)  # [128, 130]


def _patch_act_tables():
    """Make Ln and Exp both first-match to natural_log_exp_and_others so the
    whole kernel needs a single ACT table load. Set IDs stay canonical (we
    only edit membership of the shadowing sets, keyed by name)."""
    import concourse.bacc as bacc_mod
    from concourse import mybir
    if getattr(bacc_mod, "_awpa_tables_patched", False):
        return
    orig = bacc_mod.get_activation_tables
    AF = mybir.ActivationFunctionType

    def patched(arch):
        tables = dict(orig(arch))
        if "natural_log_exp_and_others" in tables:
            for name, fns in tables.items():
                if name == "natural_log_exp_and_others":
                    continue
                if AF.Ln in fns or AF.Exp in fns:
                    tables[name] = fns - {AF.Ln, AF.Exp}
        return tables

    bacc_mod.get_activation_tables = patched
    bacc_mod._awpa_tables_patched = True


def _build():
    import concourse.bass as bass
    import concourse.tile as tile
    from concourse import bacc, mybir

    _patch_act_tables()
    f32 = mybir.dt.float32
    bf16 = mybir.dt.bfloat16
    AF = mybir.ActivationFunctionType

    nc = bacc.Bacc("TRN2", target_bir_lowering=False, debug=False,
                   num_devices=NCORES)
    Qs = nc.dram_tensor("Qs", [QBLK, H], f32, kind="ExternalInput").ap()
    K = nc.dram_tensor("K", [S, H], f32, kind="ExternalInput").ap()
    V = nc.dram_tensor("V", [S, H], f32, kind="ExternalInput").ap()
    CD = nc.inline_tensor(_dft_consts(), "CDconst").ap()  # [128, 130]
    IDT = nc.inline_tensor(np.eye(H, dtype=np.float32), "IDTconst").ap()
    ow_dt = bf16 if WEIGHTS_BF16 else f32
    OW = nc.dram_tensor("OW", [QBLK, S], ow_dt, kind="ExternalOutput").ap()
    OO = nc.dram_tensor("OO", [H, QBLK], f32, kind="ExternalOutput").ap()

    with ExitStack() as ctx:
        tc = ctx.enter_context(tile.TileContext(nc))
        consts = ctx.enter_context(tc.tile_pool(name="consts", bufs=1))
        big = ctx.enter_context(tc.tile_pool(name="big", bufs=1))
        ftmp = ctx.enter_context(tc.tile_pool(name="ftmp", bufs=1))

        # ACT table preload: first ACT op is Ln -> loads the (patched) set
        # covering Ln/Exp/Square/Copy once, overlapping the input DMAs.
        junk = consts.tile([128, 1], f32)
        nc.vector.memset(junk[:], 1.0)
        junk2 = consts.tile([128, 1], f32)
        nc.scalar.activation(junk2[:], junk[:], AF.Ln)

        # --- DMA inputs (criticality order) ---
        kn = big.tile([128, NJT, H], f32)
        nc.sync.dma_start(out=kn[:], in_=K.rearrange("(t p) h -> p t h", p=128))
        idt = consts.tile([H, H], f32)
        nc.sync.dma_start(out=idt[:], in_=IDT[:])
        qn = big.tile([128, NIT, H], f32)
        nc.sync.dma_start(out=qn[:], in_=Qs.rearrange("(t p) h -> p t h", p=128))
        cdf = consts.tile([H, 2 * F], f32)
        nc.sync.dma_start(out=cdf[:], in_=CD[:])
        vn = big.tile([128, NJT, H], f32)
        nc.sync.dma_start(out=vn[:], in_=V.rearrange("(t p) h -> p t h", p=128))
        cdb = consts.tile([H, 2 * F], bf16)
        nc.vector.tensor_copy(cdb[:], cdf[:])
        vb = big.tile([128, NJT, H], bf16)
        nc.vector.tensor_copy(vb[:], vn[:])

        # --- input transposes (h on partitions) via PE, chunked copies ---
        qtb = big.tile([128, QBLK], bf16)
        ktb = big.tile([128, S], bf16)
        with tc.tile_pool(name="psA", bufs=1, space="PSUM") as psA:
            qt_ps = psA.tile([128, QBLK], f32, tag="qt")
            for t in range(NIT):
                nc.tensor.transpose(qt_ps[:, t * 128:(t + 1) * 128],
                                    qn[:, t, :], idt[:])
            nc.scalar.copy(qtb[:], qt_ps[:])
            kt_ps = psA.tile([128, S], f32, tag="kt")
            # copy pieces aligned with the DFT source chunks below
            for t0, t1 in ((0, 2), (2, 6), (6, 8)):
                for t in range(t0, t1):
                    nc.tensor.transpose(kt_ps[:, t * 128:(t + 1) * 128],
                                        kn[:, t, :], idt[:])
                nc.scalar.copy(ktb[:, t0 * 128:t1 * 128],
                               kt_ps[:, t0 * 128:t1 * 128])

        # --- DFT + features; columns [Q | K], two pipeline chunks ---
        # chunk 0 = cols [0:768] (Q + K[0:512]);  chunk 1 = [768:1280]
        uF = big.tile([F, NF], bf16)
        vF = big.tile([F - 1, NF], bf16)
        wF = big.tile([F, NF], bf16)
        with tc.tile_pool(name="psB", bufs=1, space="PSUM") as psB:
            re_ps = psB.tile([F, NF], f32, tag="re")
            im_ps = psB.tile([F, NF], f32, tag="im")
            # each matmul destination must stay inside one 2KB PSUM bank
            dft_src = ((0, QBLK, qtb, 0), (QBLK, 512, ktb, 0),
                       (512, 1024, ktb, 256), (1024, NF, ktb, 768))
            for c0, c1, src, s0 in dft_src:
                nc.tensor.matmul(re_ps[:, c0:c1], cdb[:, 0:F],
                                 src[:, s0:s0 + (c1 - c0)],
                                 start=True, stop=True)
                nc.tensor.matmul(im_ps[:, c0:c1], cdb[:, F:2 * F],
                                 src[:, s0:s0 + (c1 - c0)],
                                 start=True, stop=True)
            sqre = ftmp.tile([F, NF], bf16)
            sqim = ftmp.tile([F, NF], bf16)
            amp2 = ftmp.tile([F, NF], bf16)
            lg = ftmp.tile([F, NF], f32)
            tq = ftmp.tile([F, NF], f32)
            for c0, c1 in ((0, 768), (768, NF)):
                cc = slice(c0, c1)
                nc.scalar.activation(sqre[:, cc], re_ps[:, cc], AF.Square)
                nc.scalar.activation(sqim[:, cc], im_ps[:, cc], AF.Square)
                nc.vector.tensor_add(amp2[:, cc], sqre[:, cc], sqim[:, cc])
                nc.scalar.activation(lg[:, cc], amp2[:, cc], AF.Ln)
                nc.scalar.activation(tq[:, cc], lg[:, cc], AF.Exp, scale=-0.25)
                nc.scalar.activation(wF[:, cc], lg[:, cc], AF.Exp, scale=0.25)
                nc.vector.tensor_mul(uF[:, cc], re_ps[:, cc], tq[:, cc])
                nc.vector.tensor_mul(vF[:, cc], im_ps[0:F - 1, cc],
                                     tq[0:F - 1, cc])

        # --- scores + softmax per (query sub-tile, key half) ---
        wb_tiles = []
        with tc.tile_pool(name="psC", bufs=2, space="PSUM") as psC:
            for it in range(NIT):
                i0, i1 = it * 128, (it + 1) * 128
                e = ftmp.tile([128, S], bf16, tag=f"e{it}")
                se = []
                for hf in range(2):
                    k0 = QBLK + hf * 512  # K columns in feature tensors
                    num_ps = psC.tile([128, 512], f32, tag="num")
                    den_ps = psC.tile([128, 512], f32, tag="den")
                    nc.tensor.matmul(num_ps[:], uF[:, i0:i1],
                                     uF[:, k0:k0 + 512], start=True, stop=False)
                    nc.tensor.matmul(num_ps[:], vF[:, i0:i1],
                                     vF[:, k0:k0 + 512], start=False, stop=True)
                    nc.tensor.matmul(den_ps[:], wF[:, i0:i1],
                                     wF[:, k0:k0 + 512], start=True, stop=True)
                    inv = ftmp.tile([128, 512], f32, tag="inv")
                    nc.vector.reciprocal_approx_fast(out=inv[:], in_=den_ps[:])
                    pa = ftmp.tile([128, 512], f32, tag="pa")
                    nc.vector.tensor_mul(pa[:], num_ps[:], inv[:])
                    sh = ftmp.tile([128, 1], f32, tag=f"sume{it}{hf}")
                    nc.scalar.activation(e[:, hf * 512:(hf + 1) * 512], pa[:],
                                         AF.Exp, bias=1.0, accum_out=sh[:])
                    se.append(sh)
                sumexp = ftmp.tile([128, 1], f32, tag=f"sumexp{it}")
                nc.vector.tensor_add(sumexp[:], se[0][:], se[1][:])
                r = ftmp.tile([128, 1], f32, tag=f"r{it}")
                nc.vector.reciprocal(r[:], sumexp[:])
                wb = big.tile([128, S], bf16, tag=f"wb{it}")
                nc.vector.tensor_scalar_mul(wb[:], e[:], r[:])
                wb_tiles.append(wb)
                if WEIGHTS_BF16:
                    nc.sync.dma_start(out=OW[i0:i1, :], in_=wb[:])
                else:
                    oww = ftmp.tile([128, S], f32, tag="oww")
                    nc.vector.tensor_scalar_mul(oww[:], e[:], r[:])
                    nc.sync.dma_start(out=OW[i0:i1, :], in_=oww[:])

            # --- transpose normalized weights to [j_p, i] for P@V ---
            idtb = consts.tile([H, H], bf16)
            nc.vector.tensor_copy(idtb[:], idt[:])
            et_g = []
            for g in range(2):  # 4 j-tiles per psum group, one batched copy
                wt_ps = psC.tile([128, 4 * QBLK], bf16, tag="wt")
                for lj in range(4):
                    jt = g * 4 + lj
                    for it in range(NIT):
                        nc.tensor.transpose(
                            wt_ps[:, lj * QBLK + it * 128:
                                  lj * QBLK + (it + 1) * 128],
                            wb_tiles[it][:, jt * 128:(jt + 1) * 128], idtb[:])
                et = big.tile([128, 4, QBLK], bf16, tag=f"et{g}")
                nc.vector.tensor_copy(et[:], wt_ps[:])
                et_g.append(et)

            # --- P@V: out^T[h, i] accumulated over j tiles ---
            av_ps = psC.tile([128, QBLK], f32, tag="av")
            for jt in range(NJT):
                nc.tensor.matmul(av_ps[:], vb[:, jt, :],
                                 et_g[jt // 4][:, jt % 4, :],
                                 start=(jt == 0), stop=(jt == NJT - 1))
            oo = big.tile([H, QBLK], f32)
            nc.scalar.copy(oo[:], av_ps[:])
            nc.sync.dma_start(out=OO[:], in_=oo[:])

    nc.compile()
    return nc


def _get_nc():
    if "nc" not in _CACHE:
        _CACHE["nc"] = _build()
    return _CACHE["nc"]


def kernel(Q, K, V):
    from concourse.bass_utils import run_bass_kernel_spmd

    Q = np.ascontiguousarray(np.asarray(Q, dtype=np.float32))
    K = np.ascontiguousarray(np.asarray(K, dtype=np.float32))
    V = np.ascontiguousarray(np.asarray(V, dtype=np.float32))
    nc = _get_nc()
    in_maps = []
    for c in range(NCORES):
        b, qb = c // 4, c % 4
        in_maps.append({
            "Qs": np.ascontiguousarray(Q[b, qb * QBLK:(qb + 1) * QBLK]),
            "K": K[b],
            "V": V[b],
        })
    res = run_bass_kernel_spmd(nc, in_maps, core_ids=list(range(NCORES)))
    output = np.empty((B, S, H), np.float32)
    weights = np.empty((B, S, S), np.float32)
    for c in range(NCORES):
        b, qb = c // 4, c % 4
        rr = res.results[c]
        weights[b, qb * QBLK:(qb + 1) * QBLK, :] = rr["OW"].astype(np.float32)
        output[b, qb * QBLK:(qb + 1) * QBLK, :] = rr["OO"].T
    return output, weights


if __name__ == "__main__":
    rng = np.random.default_rng(0)
    Q = rng.standard_normal((B, S, H)).astype(np.float32)
    K = rng.standard_normal((B, S, H)).astype(np.float32)
    V = rng.standard_normal((B, S, H)).astype(np.float32)
    out, w = kernel(Q, K, V)
    print("kernel ran:", out.shape, w.shape)


# revision 18
# speedup vs baseline: 1.0741x; 1.0626x over previous
"""AmplitudeWeightedPhaseAttention Trainium2 kernel (8 NeuronCores, SPMD).

Raw Block implementation (no TileContext): per-engine instruction streams
with hand-placed coarse semaphores — avoids Tile's per-op semaphore
overhead and its ~9us kernel-tail barrier.

Math: the reference's [B,Sq,Sk,F] tensor collapses algebraically.
With rfft bin features re/im and amp2 = re^2 + im^2:
    t  = amp2^(-1/4)       u = re*t   v = im*t   w = amp2^(+1/4)
    num[i,j] = sum_f u_q u_k + v_q v_k        (v==0 at f=0 and f=64)
    den[i,j] = sum_f w_q w_k                  (rank-65 matmul)
    weights  = softmax_j(num/den + 1)         out = weights @ V
Sharding: core c owns batch c//4, query rows (c%4)*256..+256.  Scores in
natural [i_p, j] layout (softmax norm = per-partition scalar); normalized
weights are PE-transposed to feed P@V.

PSUM bank plan (stack-scoped):
  phase1: qt[0] kt[1-2]     phase2: re[0-2] im[3-5]
  phase3: sc0[0-1] sc1[2-3] wt0[4] wt1[5] av[6]
Cross-phase aliasing is ordered by the semaphore ledger (see waits marked
"bank reuse").
"""

import numpy as np

B, S, H = 2, 1024, 128
F = H // 2 + 1  # 65
NCORES = 8
QBLK = S // 4
NJT = S // 128
NIT = QBLK // 128
NF = QBLK + S  # feature cols: Q [0:256], K [256:1280]

WEIGHTS_BF16 = True

_CACHE = {}


def _dft_consts():
    h = np.arange(H, dtype=np.float64)[:, None]
    f = np.arange(F, dtype=np.float64)[None, :]
    C = np.cos(2 * np.pi * h * f / H)
    Sn = -np.sin(2 * np.pi * h * f / H)
    Sn[:, 0] = 0.0
    Sn[:, F - 1] = 0.0
    return np.concatenate([C, Sn], axis=1).astype(np.float32)  # [128, 130]


def _patch_act_tables():
    """Make Ln and Exp both first-match to natural_log_exp_and_others so one
    ACT table load covers the whole kernel (set IDs stay canonical)."""
    import concourse.bacc as bacc_mod
    from concourse import mybir
    if getattr(bacc_mod, "_awpa_tables_patched", False):
        return
    orig = bacc_mod.get_activation_tables
    AF = mybir.ActivationFunctionType

    def patched(arch):
        tables = dict(orig(arch))
        if "natural_log_exp_and_others" in tables:
            for name, fns in tables.items():
                if name != "natural_log_exp_and_others" and \
                        (AF.Ln in fns or AF.Exp in fns):
                    tables[name] = fns - {AF.Ln, AF.Exp}
        return tables

    bacc_mod.get_activation_tables = patched
    bacc_mod._awpa_tables_patched = True


class Led:
    """Pre-declared cumulative semaphore ledger. Event values are fixed by
    the declared order; inc() verifies emission matches the plan."""

    def __init__(self, sems, orders):
        self.sems = sems
        self.ev = {}
        self.next = {}
        self.plan = {}
        for s, seq in orders.items():
            c = 0
            self.plan[s] = list(seq)
            for name, by in seq:
                c += by
                self.ev[name] = (s, c)
            self.next[s] = 0

    def inc(self, instr, s, name):
        want, by = self.plan[s][self.next[s]]
        assert want == name, f"sem {s}: expected {want}, got {name}"
        self.next[s] += 1
        instr.then_inc(self.sems[s], by)

    def wait(self, eng, name):
        s, n = self.ev[name]
        eng.wait_ge(self.sems[s], n)


def _build():
    import concourse.bass as bass
    from concourse import bacc, mybir

    _patch_act_tables()
    f32 = mybir.dt.float32
    bf16 = mybir.dt.bfloat16
    AF = mybir.ActivationFunctionType

    nc = bacc.Bacc("TRN2", target_bir_lowering=False, debug=False,
                   num_devices=NCORES)
    Qs = nc.dram_tensor("Qs", [QBLK, H], f32, kind="ExternalInput").ap()
    Kd = nc.dram_tensor("K", [S, H], f32, kind="ExternalInput").ap()
    Vd = nc.dram_tensor("V", [S, H], f32, kind="ExternalInput").ap()
    IDTCD = nc.inline_tensor(np.concatenate(
        [np.eye(H, dtype=np.float32), _dft_consts()], axis=1), "IDTCD").ap()
    ow_dt = bf16 if WEIGHTS_BF16 else f32
    OW = nc.dram_tensor("OW", [QBLK, S], ow_dt, kind="ExternalOutput").ap()
    OO = nc.dram_tensor("OO", [H, QBLK], f32, kind="ExternalOutput").ap()

    def sbuf(name, shape, dt):
        return nc.alloc_sbuf_tensor(name, list(shape), dt).ap()

    junk = sbuf("junk", [128, 1], f32)
    junk2 = sbuf("junk2", [128, 1], f32)
    idtcd = sbuf("idtcd", [H, H + 2 * F], f32)
    idt = idtcd[:, 0:H]
    cdf = idtcd[:, H:H + 2 * F]
    idtb = sbuf("idtb", [H, H], bf16)
    cdb = sbuf("cdb", [H, 2 * F], bf16)
    kn = sbuf("kn", [128, NJT, H], f32)
    qn = sbuf("qn", [128, NIT, H], f32)
    vn = sbuf("vn", [128, NJT, H], f32)
    vb = sbuf("vb", [128, NJT, H], bf16)
    xtb = sbuf("xtb", [128, NF], bf16)    # transposed [Q | K], h on partitions
    uF = sbuf("uF", [F, NF], bf16)
    vF = sbuf("vF", [F - 1, NF], bf16)
    wF = sbuf("wF", [F, NF], bf16)
    sqre = sbuf("sqre", [F, NF], bf16)
    sqim = sbuf("sqim", [F, NF], bf16)
    amp2 = sbuf("amp2", [F, NF], bf16)
    lg = sbuf("lg", [F, NF], f32)
    tq = sbuf("tq", [F, NF], f32)
    inv = sbuf("inv", [128, 512], f32)
    pa = [sbuf(f"pa{i}", [128, 512], f32) for i in range(2)]
    e_t = [sbuf(f"e{i}", [128, S], bf16) for i in range(NIT)]
    sume = [sbuf(f"sume{r}", [128, 1], f32) for r in range(4)]
    sumx = [sbuf(f"sumx{i}", [128, 1], f32) for i in range(NIT)]
    rr_ = [sbuf(f"rr{i}", [128, 1], f32) for i in range(NIT)]
    wb = [sbuf(f"wb{i}", [128, S], bf16) for i in range(NIT)]
    et = [sbuf(f"etg{g}", [128, 4, QBLK], bf16) for g in range(2)]
    oo = sbuf("oo", [H, QBLK], f32)

    orders = {
        "d": [("kn", 16), ("qn", 16), ("idtcd", 16)],
        "e": [("vn", 16)],
        "o": [("owd0", 16), ("owd1", 16), ("ood", 16)],
        "p": [("qt", 1), ("ktA", 1), ("ktB", 1), ("dftA", 1), ("dftB", 1),
              ("dftC", 1), ("mm0", 1), ("mm1", 1), ("mm2", 1), ("mm3", 1),
              ("tg0", 1), ("tg1", 1), ("av0", 1), ("av1", 1)],
        "a": [("p1", 1), ("p2", 1), ("p3", 1), ("sq0", 1), ("sq1", 1),
              ("sq2", 1), ("t0", 1), ("w0", 1), ("t1", 1), ("w1", 1),
              ("t2", 1), ("w2", 1), ("e0", 1), ("e1", 1), ("e2", 1),
              ("e3", 1), ("oo", 1)],
        "v": [("junk", 1), ("idtb", 1), ("cdb", 1), ("vb", 1), ("amp2_0", 1),
              ("uv0", 1), ("amp2_1", 1), ("uv1", 1), ("amp2_2", 1),
              ("uv2", 1), ("pa0", 1), ("pa1", 1), ("pa2", 1), ("pa3", 1),
              ("wb0", 1), ("wb1", 1), ("et0", 1), ("et1", 1)],
    }

    with nc.Block(no_gpsimd_drain=True) as block, \
         nc.semaphore("sd") as sem_d, nc.semaphore("se") as sem_e, \
         nc.semaphore("so") as sem_o, nc.semaphore("sp") as sem_p, \
         nc.semaphore("sa") as sem_a, nc.semaphore("sv") as sem_v:
        L = Led({"d": sem_d, "e": sem_e, "o": sem_o, "p": sem_p,
                 "a": sem_a, "v": sem_v}, orders)
        CH = ((0, 512), (512, 1024), (1024, NF))  # feature chunks

        # ---------------- phase 1: input DMA, casts, transposes ----------
        with nc.psum_tensor("qt_ps", [128, QBLK], f32) as qt_h, \
             nc.psum_tensor("kt_ps", [128, S], f32) as kt_h:
            qt_ps, kt_ps = qt_h.ap(), kt_h.ap()

            @block.sync
            def _(sp):
                L.inc(sp.dma_start(out=kn[:], in_=Kd.rearrange(
                    "(t p) h -> p t h", p=128)), "d", "kn")
                L.inc(sp.dma_start(out=qn[:], in_=Qs.rearrange(
                    "(t p) h -> p t h", p=128)), "d", "qn")
                L.inc(sp.dma_start(out=idtcd[:], in_=IDTCD[:]),
                      "d", "idtcd")
                L.inc(sp.dma_start(out=vn[:], in_=Vd.rearrange(
                    "(t p) h -> p t h", p=128)), "e", "vn")

            @block.vector
            def _(dv):
                L.inc(dv.memset(junk[:], 1.0), "v", "junk")
                L.wait(dv, "idtcd")
                L.inc(dv.tensor_copy(idtb[:], idt[:]), "v", "idtb")
                L.inc(dv.tensor_copy(cdb[:], cdf[:]), "v", "cdb")
                L.wait(dv, "vn")
                L.inc(dv.tensor_copy(vb[:], vn[:]), "v", "vb")

            @block.tensor
            def _(pe):
                L.wait(pe, "idtcd")  # kn, qn land earlier in queue order
                mi = None
                for t in range(NIT):
                    mi = pe.matmul(qt_ps[:, t * 128:(t + 1) * 128],
                                   qn[:, t, :], idt[:], is_transpose=True,
                                   start=True, stop=True)
                L.inc(mi, "p", "qt")
                for t0, t1, evn in ((0, 4, "ktA"), (4, 8, "ktB")):
                    for t in range(t0, t1):
                        mi = pe.matmul(kt_ps[:, t * 128:(t + 1) * 128],
                                       kn[:, t, :], idt[:], is_transpose=True,
                                       start=True, stop=True)
                    L.inc(mi, "p", evn)

            @block.scalar
            def _(sc):
                L.wait(sc, "junk")
                sc.activation(junk2[:], junk[:], AF.Ln)  # ACT table preload
                L.wait(sc, "qt")
                L.inc(sc.copy(xtb[:, 0:QBLK], qt_ps[:]), "a", "p1")
                L.wait(sc, "ktA")
                L.inc(sc.copy(xtb[:, QBLK:QBLK + 512], kt_ps[:, 0:512]),
                      "a", "p2")
                L.wait(sc, "ktB")
                L.inc(sc.copy(xtb[:, QBLK + 512:NF], kt_ps[:, 512:1024]),
                      "a", "p3")

        # ---------------- phase 2: DFT + features -----------------------
        with nc.psum_tensor("re_ps", [F, NF], f32) as re_h, \
             nc.psum_tensor("im_ps", [F, NF], f32) as im_h:
            re_ps, im_ps = re_h.ap(), im_h.ap()

            @block.tensor
            def _(pe):
                L.wait(pe, "cdb")
                # bank-reuse note: re[0-2]/im[3-5] overlap qt[0]+kt[1-2];
                # the p2/p3 waits below also order those reads before writes.
                for (c0, c1), evp, evn in ((CH[0], "p2", "dftA"),
                                           (CH[1], "p3", "dftB"),
                                           (CH[2], "p3", "dftC")):
                    L.wait(pe, evp)
                    pe.matmul(re_ps[:, c0:c1], cdb[:, 0:F], xtb[:, c0:c1],
                              start=True, stop=True)
                    mi = pe.matmul(im_ps[:, c0:c1], cdb[:, F:2 * F],
                                   xtb[:, c0:c1], start=True, stop=True)
                    L.inc(mi, "p", evn)

            @block.scalar
            def _(sc):
                for X, dep in ((0, "dftA"), (1, "dftB"), (2, "dftC")):
                    c0, c1 = CH[X]
                    L.wait(sc, dep)
                    sc.activation(sqre[:, c0:c1], re_ps[:, c0:c1], AF.Square)
                    L.inc(sc.activation(sqim[:, c0:c1], im_ps[:, c0:c1],
                                        AF.Square), "a", f"sq{X}")
                for X in (0, 1, 2):
                    c0, c1 = CH[X]
                    L.wait(sc, f"amp2_{X}")
                    sc.activation(lg[:, c0:c1], amp2[:, c0:c1], AF.Ln)
                    sc.drain()
                    L.inc(sc.activation(tq[:, c0:c1], lg[:, c0:c1], AF.Exp,
                                        scale=-0.25), "a", f"t{X}")
                    L.inc(sc.activation(wF[:, c0:c1], lg[:, c0:c1], AF.Exp,
                                        scale=0.25), "a", f"w{X}")

            @block.vector
            def _(dv):
                for X in (0, 1, 2):
                    c0, c1 = CH[X]
                    L.wait(dv, f"sq{X}")
                    L.inc(dv.tensor_add(amp2[:, c0:c1], sqre[:, c0:c1],
                                        sqim[:, c0:c1]), "v", f"amp2_{X}")
                    L.wait(dv, f"t{X}")
                    dv.tensor_mul(uF[:, c0:c1], re_ps[:, c0:c1], tq[:, c0:c1])
                    L.inc(dv.tensor_mul(vF[:, c0:c1],
                                        im_ps[0:F - 1, c0:c1],
                                        tq[0:F - 1, c0:c1]), "v", f"uv{X}")

        # ---------------- phase 3: scores, softmax, P@V -----------------
        # rounds: r0=(it0,hfA,sc0) r1=(it1,hfA,sc1) r2=(it0,hfB,sc0)
        #         r3=(it1,hfB,sc1);  num = slot[:,0:512], den = [:,512:1024]
        with nc.psum_tensor("sc0", [128, S], f32) as s0h, \
             nc.psum_tensor("sc1", [128, S], f32) as s1h, \
             nc.psum_tensor("wt0", [128, 8 * 128], bf16) as w0h, \
             nc.psum_tensor("wt1", [128, 8 * 128], bf16) as w1h, \
             nc.psum_tensor("av_ps", [128, QBLK], f32) as avh:
            scp = [s0h.ap(), s1h.ap()]
            wt = [w0h.ap(), w1h.ap()]
            av_ps = avh.ap()
            rounds = [(0, 0), (1, 0), (0, 1), (1, 1)]  # (it, hf); slot=it%2

            @block.tensor
            def _(pe):
                for r, (it, hf) in enumerate(rounds):
                    i0, i1 = it * 128, (it + 1) * 128
                    k0 = QBLK + hf * 512
                    slot = scp[r % 2]
                    if r == 0:
                        L.wait(pe, "uv1")   # features for hfA rhs
                        L.wait(pe, "w1")
                    elif r == 1:
                        L.wait(pe, "uv2")   # bank reuse: sc1[2-3] aliases
                        L.wait(pe, "w2")    # re[2]/im[3] (chunk C / chunk A)
                    else:
                        L.wait(pe, f"pa{r - 2}")  # sc slot reuse
                    pe.matmul(slot[:, 0:512], uF[:, i0:i1],
                              uF[:, k0:k0 + 512], start=True, stop=False)
                    pe.matmul(slot[:, 0:512], vF[:, i0:i1],
                              vF[:, k0:k0 + 512], start=False, stop=True)
                    L.inc(pe.matmul(slot[:, 512:1024], wF[:, i0:i1],
                                    wF[:, k0:k0 + 512], start=True,
                                    stop=True), "p", f"mm{r}")
                # weight transposes (wt banks 4-5 alias im[4-5]: the uv2 wait
                # above already ordered those reads), then P@V
                for g in range(2):
                    mi = None
                    for it in range(NIT):
                        L.wait(pe, f"wb{it}")
                        for lj in range(4):
                            jt = g * 4 + lj
                            mi = pe.matmul(
                                wt[g][:, lj * 2 * 128 + it * 128:
                                      lj * 2 * 128 + (it + 1) * 128],
                                wb[it][:, jt * 128:(jt + 1) * 128], idtb[:],
                                is_transpose=True, start=True, stop=True)
                    L.inc(mi, "p", f"tg{g}")
                for g in range(2):
                    L.wait(pe, f"et{g}")
                    mi = None
                    for lj in range(4):
                        jt = g * 4 + lj
                        mi = pe.matmul(av_ps[:], vb[:, jt, :],
                                       et[g][:, lj, :],
                                       start=(jt == 0), stop=(jt == NJT - 1))
                    L.inc(mi, "p", f"av{g}")

            @block.vector
            def _(dv):
                for r, (it, hf) in enumerate(rounds):
                    L.wait(dv, f"mm{r}")
                    if r >= 2:  # pa buffer reuse: ACT must have read it
                        L.wait(dv, f"e{r - 2}")
                    slot = scp[r % 2]
                    dv.drain()
                    dv.reciprocal_approx_fast(out=inv[:],
                                              in_=slot[:, 512:1024])
                    dv.drain()
                    L.inc(dv.tensor_mul(pa[r % 2][:], slot[:, 0:512],
                                        inv[:]), "v", f"pa{r}")
                for it in range(NIT):
                    L.wait(dv, f"e{it + 2}")  # r2 -> it0, r3 -> it1
                    dv.tensor_add(sumx[it][:], sume[it][:], sume[it + 2][:])
                    dv.drain()
                    dv.reciprocal(rr_[it][:], sumx[it][:])
                    dv.drain()
                    L.inc(dv.tensor_scalar_mul(wb[it][:], e_t[it][:],
                                               rr_[it][:]), "v", f"wb{it}")
                for g in range(2):
                    L.wait(dv, f"tg{g}")
                    L.inc(dv.tensor_copy(et[g][:], wt[g][:]), "v", f"et{g}")

            @block.scalar
            def _(sc):
                for r, (it, hf) in enumerate(rounds):
                    L.wait(sc, f"pa{r}")
                    L.inc(sc.activation(e_t[it][:, hf * 512:(hf + 1) * 512],
                                        pa[r % 2][:], AF.Exp, bias=1.0,
                                        accum_out=sume[r][:]), "a", f"e{r}")
                L.wait(sc, "av1")
                L.inc(sc.copy(oo[:], av_ps[:]), "a", "oo")

            @block.sync
            def _(sp):
                for it in range(NIT):
                    L.wait(sp, f"wb{it}")
                    L.inc(sp.dma_start(out=OW[it * 128:(it + 1) * 128, :],
                                       in_=wb[it][:]), "o", f"owd{it}")
                L.wait(sp, "oo")
                L.inc(sp.dma_start(out=OO[:], in_=oo[:]), "o", "ood")
                L.wait(sp, "ood")

    nc.compile()
    return nc


def _get_nc():
    if "nc" not in _CACHE:
        _CACHE["nc"] = _build()
    return _CACHE["nc"]


def kernel(Q, K, V):
    from concourse.bass_utils import run_bass_kernel_spmd

    Q = np.ascontiguousarray(np.asarray(Q, dtype=np.float32))
    K = np.ascontiguousarray(np.asarray(K, dtype=np.float32))
    V = np.ascontiguousarray(np.asarray(V, dtype=np.float32))
    nc = _get_nc()
    in_maps = []
    for c in range(NCORES):
        b, qb = c // 4, c % 4
        in_maps.append({
            "Qs": np.ascontiguousarray(Q[b, qb * QBLK:(qb + 1) * QBLK]),
            "K": K[b],
            "V": V[b],
        })
    res = run_bass_kernel_spmd(nc, in_maps, core_ids=list(range(NCORES)))
    output = np.empty((B, S, H), np.float32)
    weights = np.empty((B, S, S), np.float32)
    for c in range(NCORES):
        b, qb = c // 4, c % 4
        rr = res.results[c]
        weights[b, qb * QBLK:(qb + 1) * QBLK, :] = rr["OW"].astype(np.float32)
        output[b, qb * QBLK:(qb + 1) * QBLK, :] = rr["OO"].T
    return output, weights


if __name__ == "__main__":
    rng = np.random.default_rng(0)
    Q = rng.standard_normal((B, S, H)).astype(np.float32)
    K = rng.standard_normal((B, S, H)).astype(np.float32)
    V = rng.standard_normal((B, S, H)).astype(np.float32)
    out, w = kernel(Q, K, V)
    print("kernel ran:", out.shape, w.shape)
